# revision 1
# baseline (speedup 1.0000x reference)
"""Trainium2 Bass kernel for nn_Attention_73375221285454.

Multi-head self-attention (B=4, N=2048, D=768, H=12, DH=64) with key-padding
mask, distributed over 8 NeuronCores.

Sharding: core c handles batch b = c//2 and query half qh = c%2 (1024 query
rows). Each core computes K/V for its full batch (duplicated across the pair)
and attention + output projection for its query half; the 8 outputs tile the
full (4, 2048, 768) result with no collectives.

Host marshalling per core: x[b] is transposed (xkT for keys — sorted so that
unmasked keys come first, making trailing all-masked key tiles skippable —
and xqT for the query half in natural order); the bool mask becomes float
additive/multiplicative mask tables. Attention is permutation-invariant over
keys, so sorting keys (with the mask sorted identically) is exact.

Device algorithm per core (all matmuls in float32r ~ tf32):
  V    = (xkT.T @ Wv) stored as vaug [128, 16, 12, 65] with a ones column
  K^T  = Wk.T @ xkT  -> kT [128, 6, njt*128]    (only active key tiles)
  Q^T  = Wq.T @ xqT  -> qT [128, 6, 1024]
  per head h, active key tile jt:
    S^T[j, i] = K_h^T.T @ Q_h^T                (PSUM [128, 1024])
    P^T       = exp(0.125*S^T + cmneg[j])      (ACT; cmneg=-30000 if masked)
    O^T      += vaug[jt, h].T @ P^T            (PSUM [65, 1024]; row 64 = s[i])
  attnT_h = O^T[0:64] staged unnormalized; s-rows gathered into [12, 1024];
  one batched reciprocal, then per-head rank-1 (ones (x) 1/s) via PE and an
  in-place multiply normalizes attnT.
  out     = (attnT.T @ Wo) * rm01[i] + (1 - rm01[i]) (x) uniform_row
  where uniform_row = (mean_all_keys V) @ Wo reproduces the reference's
  uniform softmax over ALL keys for fully-masked query rows.

No max-subtraction is needed: logits are ~N(0,1) (exp can't overflow), masked
keys get exp(logit - 30000) == 0 exactly, and fully-masked query rows are
replaced by uniform_row at the end.
"""

import sys

sys.path.insert(0, "/opt/trn_rl_repo")

import numpy as np

import concourse.bass as bass  # noqa: F401
import concourse.mybir as mybir
import concourse.tile as tile
from concourse.tile import add_dep_helper
from concourse import bacc
from concourse.bass_utils import run_bass_kernel_spmd

P = 128
B, N, D = 4, 2048, 768
H, DH = 12, 64
NQ = N // 2              # queries per core
DC = D // P              # 6 contraction chunks
NJT_FULL = N // P        # 16 key tiles
NIT = NQ // P            # 8 query tiles
SCALE = DH ** -0.5       # 0.125
MASK_NEG = -30000.0
SORT_KEYS = True         # sort keys so all-masked key tiles are skipped

f32 = mybir.dt.float32
f32r = mybir.dt.float32r

_BUILD_CACHE = {}


def build(njt_act: int) -> "bacc.Bacc":
    """Build the SPMD program. njt_act = number of key tiles containing any
    unmasked key; trailing all-masked tiles contribute exactly zero to both
    softmax numerator and denominator and are skipped. V/meanV still cover
    all 16 tiles (masked-query rows need the mean over ALL keys)."""
    if njt_act in _BUILD_CACHE:
        return _BUILD_CACHE[njt_act]

    nk = njt_act * P  # active key columns

    nc = bacc.Bacc()
    xkT_d = nc.declare_dram_parameter("xkT", [D, N], f32, isOutput=False)
    xqT_d = nc.declare_dram_parameter("xqT", [D, NQ], f32, isOutput=False)
    wq_d = nc.declare_dram_parameter("Wq", [D, D], f32, isOutput=False)
    wk_d = nc.declare_dram_parameter("Wk", [D, D], f32, isOutput=False)
    wv_d = nc.declare_dram_parameter("Wv", [D, D], f32, isOutput=False)
    wo_d = nc.declare_dram_parameter("Wo", [D, D], f32, isOutput=False)
    # cmnegT[p, t] = 0.0 if key (t*128+p) unmasked else -30000.0
    cmneg_d = nc.declare_dram_parameter("cmnegT", [P, NJT_FULL], f32, isOutput=False)
    # rm01T[p, t] = 1.0 if query (t*128+p) unmasked else 0.0
    rm01_d = nc.declare_dram_parameter("rm01T", [P, NIT], f32, isOutput=False)
    # rmneg_row[0, i] = 1.0 - rm01[i]
    rmneg_d = nc.declare_dram_parameter("rmneg_row", [1, NQ], f32, isOutput=False)
    out_d = nc.declare_dram_parameter("out", [NQ, D], f32, isOutput=True)

    xkT_r = xkT_d.rearrange("(c p) n -> p c n", p=P).bitcast(f32r)
    xqT_r = xqT_d.rearrange("(c p) n -> p c n", p=P).bitcast(f32r)
    wv_r = wv_d.rearrange("(c p) e -> p c e", p=P).bitcast(f32r)
    wq_r = wq_d.rearrange("(c p) e -> p c e", p=P).bitcast(f32r)
    wk_r = wk_d.rearrange("(c p) e -> p c e", p=P).bitcast(f32r)
    wo_r = wo_d.rearrange("(c p) e -> p c e", p=P).bitcast(f32r)

    with tile.TileContext(nc) as tc:
        with tc.tile_pool(name="persist", bufs=1) as persist:
            # small persistent tiles
            cmneg = persist.tile([P, NJT_FULL], f32)
            nc.sync.dma_start(out=cmneg, in_=cmneg_d.ap())
            rm01 = persist.tile([P, NIT], f32)
            nc.sync.dma_start(out=rm01, in_=rm01_d.ap())
            rmneg_row = persist.tile([1, NQ], f32r)
            nc.sync.dma_start(out=rmneg_row, in_=rmneg_d.ap().bitcast(f32r))
            ones_f = persist.tile([P, H], f32)
            nc.vector.memset(ones_f, 1.0)
            ones_r = persist.tile([P, 1], f32r)
            nc.vector.tensor_copy(ones_r, ones_f[:, 0:1])
            id1 = persist.tile([1, 1], f32)
            nc.vector.memset(id1, 1.0)

            qT = persist.tile([P, DC, NQ], f32r)
            vaug = persist.tile([P, NJT_FULL, H, DH + 2], f32r)
            kT = persist.tile([P, DC, nk], f32r)
            mvT_sb = persist.tile([P, DC], f32r)   # meanV^T (already / N)
            mv_row = persist.tile([1, D], f32)

            with tc.tile_pool(name="xk_pool", bufs=1) as xk_pool:
                # ------------- phase 1: V projection (+ meanV) -------------
                xkT = xk_pool.tile([P, DC, N], f32r)
                vproj_scope = nc.named_scope("vproj"); vproj_scope.__enter__()
                with tc.tile_pool(name="wv_pool", bufs=1) as wv_pool, \
                     tc.tile_pool(name="psp2", bufs=2, space="PSUM") as psp2, \
                     tc.tile_pool(name="psmv", bufs=1, space="PSUM") as psmv:
                    wv_sb = wv_pool.tile([P, DC, D], f32r)
                    # chunked loads so the first V matmuls start early
                    for dc in range(DC):
                        nc.sync.dma_start(out=wv_sb[:, dc, :], in_=wv_r[:, dc, :])
                    for cg in range(4):
                        for dc in range(DC):
                            nc.sync.dma_start(
                                out=xkT[:, dc, cg * 512 : (cg + 1) * 512],
                                in_=xkT_r[:, dc, cg * 512 : (cg + 1) * 512],
                            )
                    for jt in range(NJT_FULL):
                        psv = psp2.tile([P, D], f32, tag="psv")
                        for dc in range(DC):
                            nc.tensor.matmul(
                                psv[:, 0:512],
                                xkT[:, dc, jt * P : (jt + 1) * P],
                                wv_sb[:, dc, 0:512],
                                start=(dc == 0),
                                stop=(dc == DC - 1),
                            )
                        for dc in range(DC):
                            nc.tensor.matmul(
                                psv[:, 512:768],
                                xkT[:, dc, jt * P : (jt + 1) * P],
                                wv_sb[:, dc, 512:768],
                                start=(dc == 0),
                                stop=(dc == DC - 1),
                            )
                        nc.vector.tensor_copy(
                            vaug[:, jt, :, 0:DH],
                            psv.rearrange("p (h d) -> p h d", h=H),
                        )
                        nc.vector.tensor_copy(
                            vaug[:, jt, :, DH : DH + 2],
                            ones_f[:, :, None].to_broadcast([P, H, 2]),
                        )

                    # meanV over ALL keys -> mvT_sb [128, 6], scaled by 1/N
                    ps_mv = psmv.tile([1, D], f32, tag="ps_mv")
                    for jt in range(NJT_FULL):
                        nc.tensor.matmul(
                            ps_mv[:, 0:512],
                            ones_r,
                            vaug[:, jt, 0:8, 0:DH],
                            start=(jt == 0),
                            stop=(jt == NJT_FULL - 1),
                        )
                    for jt in range(NJT_FULL):
                        nc.tensor.matmul(
                            ps_mv[:, 512:768],
                            ones_r,
                            vaug[:, jt, 8:12, 0:DH],
                            start=(jt == 0),
                            stop=(jt == NJT_FULL - 1),
                        )
                    nc.vector.tensor_scalar_mul(mv_row, in0=ps_mv, scalar1=1.0 / N)
                    ps_mvt = psmv.tile([P, DC], f32, tag="ps_mvt")
                    for c in range(DC):
                        nc.tensor.transpose(
                            ps_mvt[:, c : c + 1],
                            mv_row[0:1, c * P : (c + 1) * P],
                            id1,
                        )
                    nc.vector.tensor_copy(mvT_sb, ps_mvt)

                vproj_scope.__exit__(None, None, None)
                qproj_scope = nc.named_scope("qproj"); qproj_scope.__enter__()
                # ---------------- phase 2: Q projection ----------------
                with tc.tile_pool(name="xq_pool", bufs=1) as xq_pool, \
                     tc.tile_pool(name="wst1", bufs=2) as wst1, \
                     tc.tile_pool(name="psp1", bufs=3, space="PSUM") as psp1:
                    xqT = xq_pool.tile([P, DC, NQ], f32r)
                    for dc in range(DC):
                        nc.sync.dma_start(out=xqT[:, dc, :], in_=xqT_r[:, dc, :])
                    for hdt in range(DC):
                        wq_t = wst1.tile([P, DC, P], f32r, tag="wstream")
                        nc.sync.dma_start(
                            out=wq_t, in_=wq_r[:, :, hdt * P : (hdt + 1) * P]
                        )
                        for nch in range(NQ // 512):
                            ps = psp1.tile([P, 512], f32, tag="psproj")
                            for dc in range(DC):
                                nc.tensor.matmul(
                                    ps,
                                    wq_t[:, dc, :],
                                    xqT[:, dc, nch * 512 : (nch + 1) * 512],
                                    start=(dc == 0),
                                    stop=(dc == DC - 1),
                                )
                            nc.vector.tensor_copy(
                                qT[:, hdt, nch * 512 : (nch + 1) * 512], ps
                            )

                qproj_scope.__exit__(None, None, None)
                kproj_scope = nc.named_scope("kproj"); kproj_scope.__enter__()
                # ---------------- phase 3: K projection ----------------
                with tc.tile_pool(name="wst3", bufs=2) as wst3, \
                     tc.tile_pool(name="psp3", bufs=3, space="PSUM") as psp3:
                    nch_sizes = []
                    off = 0
                    while off < nk:
                        sz = min(512, nk - off)
                        if nk - (off + sz) == 128:  # avoid a 128-wide tail
                            sz = 384
                        nch_sizes.append((off, sz))
                        off += sz
                    for hdt in range(DC):
                        wk_t = wst3.tile([P, DC, P], f32r, tag="wstream3")
                        nc.sync.dma_start(
                            out=wk_t, in_=wk_r[:, :, hdt * P : (hdt + 1) * P]
                        )
                        for off, sz in nch_sizes:
                            ps = psp3.tile([P, 512], f32, tag="psproj3")
                            for dc in range(DC):
                                nc.tensor.matmul(
                                    ps[:, 0:sz],
                                    wk_t[:, dc, :],
                                    xkT[:, dc, off : off + sz],
                                    start=(dc == 0),
                                    stop=(dc == DC - 1),
                                )
                            nc.vector.tensor_copy(
                                kT[:, hdt, off : off + sz], ps[:, 0:sz]
                            )

            kproj_scope.__exit__(None, None, None)
            attn_scope = nc.named_scope("attn"); attn_scope.__enter__()
            # ---------------- phase 4a: attention heads ----------------
            attn_pool_cm = tc.tile_pool(name="attn_pool", bufs=1)
            attn_pool = attn_pool_cm.__enter__()
            attnT = attn_pool.tile([P, DC, NQ], f32r)
            with tc.tile_pool(name="psS", bufs=4, space="PSUM") as psS_pool, \
                 tc.tile_pool(name="psO", bufs=2, space="PSUM") as psO_pool, \
                 tc.tile_pool(name="pts", bufs=3) as pts, \
                 tc.tile_pool(name="nrm", bufs=1) as nrm:
                for h in range(H):
                    hdt, hh = h // 2, h % 2
                    pbase = DH * hh
                    psO = psO_pool.tile([DH + 2, NQ], f32, tag="psO",
                                        name=f"psOh{h % 2}")
                    prev = None
                    for jt in range(njt_act + 1):
                        cur = []
                        if jt < njt_act:
                            for q2 in range(NQ // 512):
                                qsl = slice(q2 * 512, (q2 + 1) * 512)
                                psS = psS_pool.tile([P, 512], f32, tag="psS",
                                                    name=f"psS{q2}")
                                nc.tensor.matmul(
                                    psS,
                                    kT[pbase : pbase + DH, hdt,
                                       jt * P : (jt + 1) * P],
                                    qT[pbase : pbase + DH, hdt, qsl],
                                    start=True,
                                    stop=True,
                                )
                                cur.append((q2, qsl, psS))
                        if prev is not None:
                            pjt, plist = prev
                            for q2, qsl, pT in plist:
                                nc.tensor.matmul(
                                    psO[:, qsl],
                                    vaug[:, pjt, h, :],
                                    pT,
                                    start=(pjt == 0),
                                    stop=(pjt == njt_act - 1),
                                )
                        if jt < njt_act:
                            plist = []
                            for q2, qsl, psS in cur:
                                pTf = pts.tile([P, 512], f32, tag=f"pTf{q2}")
                                nc.scalar.activation(
                                    pTf,
                                    psS,
                                    mybir.ActivationFunctionType.Exp,
                                    bias=cmneg[:, jt : jt + 1],
                                    scale=SCALE,
                                )
                                pT = pts.tile([P, 512], f32r, tag=f"pT{q2}")
                                nc.vector.tensor_copy(pT, pTf.bitcast(f32r))
                                plist.append((q2, qsl, pT))
                            prev = (jt, plist)
                    # 1/s = exp(-ln(s)) on ACT (both tables in one set)
                    lns = nrm.tile([1, NQ], f32, tag="lns")
                    nc.scalar.activation(
                        lns, psO[DH : DH + 1, :],
                        mybir.ActivationFunctionType.Ln,
                    )
                    r_row = nrm.tile([1, NQ], f32r, tag=f"r_row{h % 2}")
                    nc.scalar.activation(
                        r_row, lns,
                        mybir.ActivationFunctionType.Exp, scale=-1.0,
                    )
                    # broadcast 1/s on idle GpSimd, then normalize while
                    # copying out of PSUM (inputs share start partition 0)
                    rb_sb = nrm.tile([DH, NQ], f32r, tag=f"rb_sb{h % 2}")
                    nc.gpsimd.partition_broadcast(rb_sb, r_row, channels=DH)
                    nc.vector.tensor_mul(
                        attnT[pbase : pbase + DH, hdt, :],
                        psO[0:DH, :],
                        rb_sb,
                    )
            attn_scope.__exit__(None, None, None)
            fin_scope = nc.named_scope("final"); fin_scope.__enter__()
            # -------- phase 5: output projection + masked-query fill --------
            with tc.tile_pool(name="wo_pool", bufs=1) as wo_pool, \
                 tc.tile_pool(name="fin", bufs=3) as fin, \
                 tc.tile_pool(name="psF", bufs=2, space="PSUM") as psF_pool, \
                 tc.tile_pool(name="psU", bufs=1, space="PSUM") as psU_pool:
                wo_sb = wo_pool.tile([P, DC, D], f32r)
                for dc in range(DC):
                    nc.sync.dma_start(out=wo_sb[:, dc, :], in_=wo_r[:, dc, :])
                # uniform_row = meanV @ Wo  [1, 768]
                ps_u1 = psU_pool.tile([1, D], f32, tag="ps_u1")
                for c in range(DC):
                    nc.tensor.matmul(
                        ps_u1[:, 0:512],
                        mvT_sb[:, c : c + 1],
                        wo_sb[:, c, 0:512],
                        start=(c == 0),
                        stop=(c == DC - 1),
                    )
                for c in range(DC):
                    nc.tensor.matmul(
                        ps_u1[:, 512:768],
                        mvT_sb[:, c : c + 1],
                        wo_sb[:, c, 512:768],
                        start=(c == 0),
                        stop=(c == DC - 1),
                    )
                urow_sb = fin.tile([1, D], f32r, tag="urow")
                nc.vector.tensor_copy(urow_sb, ps_u1)

                for it in range(NIT):
                    psF = psF_pool.tile([P, D], f32, tag="psF")
                    for c in range(DC):
                        nc.tensor.matmul(
                            psF[:, 0:512],
                            attnT[:, c, it * P : (it + 1) * P],
                            wo_sb[:, c, 0:512],
                            start=(c == 0),
                            stop=(c == DC - 1),
                        )
                    for c in range(DC):
                        nc.tensor.matmul(
                            psF[:, 512:768],
                            attnT[:, c, it * P : (it + 1) * P],
                            wo_sb[:, c, 512:768],
                            start=(c == 0),
                            stop=(c == DC - 1),
                        )
                    # uniform filler for masked queries: (1-rm01) (x) urow
                    psu = psU_pool.tile([P, D], f32, tag="psu")
                    nc.tensor.matmul(
                        psu[:, 0:512],
                        rmneg_row[0:1, it * P : (it + 1) * P],
                        urow_sb[0:1, 0:512],
                        start=True,
                        stop=True,
                    )
                    nc.tensor.matmul(
                        psu[:, 512:768],
                        rmneg_row[0:1, it * P : (it + 1) * P],
                        urow_sb[0:1, 512:768],
                        start=True,
                        stop=True,
                    )
                    sel_sb = fin.tile([P, D], f32, tag="sel")
                    nc.vector.tensor_scalar_mul(
                        sel_sb, in0=psF, scalar1=rm01[:, it : it + 1]
                    )
                    out_sb = fin.tile([P, D], f32, tag="outsb")
                    nc.vector.tensor_add(out_sb, sel_sb, psu)
                    nc.sync.dma_start(
                        out=out_d.ap()[it * P : (it + 1) * P, :], in_=out_sb
                    )
            fin_scope.__exit__(None, None, None)
            attn_pool_cm.__exit__(None, None, None)

    nc.compile()
    _BUILD_CACHE[njt_act] = nc
    return nc


def _marshal(x, x_mask, Wq, Wk, Wv, Wo):
    """Build per-core input maps. Returns (in_maps, njt_act)."""
    x = np.asarray(x, dtype=np.float32)
    x_mask = np.asarray(x_mask).astype(bool)
    Wq = np.ascontiguousarray(np.asarray(Wq, dtype=np.float32))
    Wk = np.ascontiguousarray(np.asarray(Wk, dtype=np.float32))
    Wv = np.ascontiguousarray(np.asarray(Wv, dtype=np.float32))
    Wo = np.ascontiguousarray(np.asarray(Wo, dtype=np.float32))

    if SORT_KEYS:
        # per-batch stable sort: unmasked keys first
        orders = [np.argsort(~x_mask[b], kind="stable") for b in range(B)]
        counts = [int(x_mask[b].sum()) for b in range(B)]
        njt_act = max(1, -(-max(counts) // P))  # ceil(max unmasked / 128)
    else:
        orders = [np.arange(N) for _ in range(B)]
        njt_act = NJT_FULL

    in_maps = []
    for c in range(8):
        b, qh = c // 2, c % 2
        order = orders[b]
        xk = x[b][order]                       # [N, D] keys (sorted)
        mk = x_mask[b][order]                  # [N] key mask (sorted)
        xq = x[b, qh * NQ : (qh + 1) * NQ]     # [NQ, D] queries natural
        mq = x_mask[b, qh * NQ : (qh + 1) * NQ]

        cm = np.where(mk, 0.0, MASK_NEG).astype(np.float32)      # [N]
        cmnegT = np.ascontiguousarray(cm.reshape(NJT_FULL, P).T)  # [128, 16]
        rm = mq.astype(np.float32)                                # [NQ]
        rm01T = np.ascontiguousarray(rm.reshape(NIT, P).T)        # [128, 8]
        rmneg_row = np.ascontiguousarray((1.0 - rm).reshape(1, NQ))

        in_maps.append({
            "xkT": np.ascontiguousarray(xk.T),   # [768, 2048]
            "xqT": np.ascontiguousarray(xq.T),   # [768, 1024]
            "Wq": Wq, "Wk": Wk, "Wv": Wv, "Wo": Wo,
            "cmnegT": cmnegT,
            "rm01T": rm01T,
            "rmneg_row": rmneg_row,
        })
    return in_maps, njt_act


def run(x, x_mask, Wq, Wk, Wv, Wo, trace=False, tmpdir=None):
    """Run on 8 cores; returns (full_output, BassKernelResults)."""
    in_maps, njt_act = _marshal(x, x_mask, Wq, Wk, Wv, Wo)
    nc = build(njt_act)
    res = run_bass_kernel_spmd(
        nc, in_maps, core_ids=list(range(8)), trace=trace, tmpdir=tmpdir
    )
    out = np.empty((B, N, D), dtype=np.float32)
    for c in range(8):
        b, qh = c // 2, c % 2
        out[b, qh * NQ : (qh + 1) * NQ] = res.results[c]["out"]
    return out, res


def kernel(**inputs) -> np.ndarray:
    out, _ = run(
        inputs["x"], inputs["x_mask"],
        inputs["Wq"], inputs["Wk"], inputs["Wv"], inputs["Wo"],
        trace=False,
    )
    return out



# revision 2
# speedup vs baseline: 2.1507x; 2.1507x over previous
"""Trainium2 Bass kernel for nn_Attention_73375221285454.

Multi-head self-attention (B=4, N=2048, D=768, H=12, DH=64) with key-padding
mask, distributed over 8 NeuronCores.

Sharding: core c handles batch b = c//2 and half of that batch's UNMASKED
query rows (qh = c%2). Each core computes K/V for its batch's unmasked keys
and attention + output projection for its query share; the 8 outputs cover
all unmasked rows. Rows with a masked query get the batch's uniform-softmax
row (mean over ALL keys of V, then @ Wo), which the host computes directly
(two 768-dim GEMVs per batch) and scatters during unsharding.

Host marshalling per core: keys sorted so unmasked keys come first (attention
is permutation-invariant over keys; the additive -30000 mask table is sorted
identically, so trailing all-masked key tiles are skipped exactly). Unmasked
queries are gathered/split between the core pair. x and all weights are cast
to bfloat16 (PE runs 1 cycle/row for bf16 vs 2+ for fp32; PSUM accumulation
stays fp32 so only operand rounding is lost; measured end-to-end max-rel
~6e-3 vs the 2e-2 gate).

Device algorithm per core (all matmul operands bf16, PSUM fp32):
  V    = (xkT.T @ Wv) stored as vaug [128, njt, 12, 66] with a ones column
  Q^T  = Wq.T @ xqT  -> qT [128, 6, nq]
  K^T  = Wk.T @ xkT  -> kT [128, 6, nk]
  per head h, active key tile jt:
    S^T[j, i] = K_h^T.T @ Q_h^T                (PSUM [128, nq])
    P^T       = exp(0.125*S^T + cmneg[j])      (ACT; bf16 out; cmneg=-30000)
    O^T      += vaug[jt, h].T @ P^T            (PSUM [66, nq]; row 64 = s[i])
  r = 1/s on DVE (vector.reciprocal), broadcast on GpSimd, normalize while
  copying out of PSUM (vector multiply, bf16 out into attnT).
  out  = attnT.T @ Wo  (fp32 out rows, DMA per query tile)

No max-subtraction is needed: logits are ~N(0,1) (exp can't overflow) and
masked keys get exp(logit - 30000) == 0 exactly.
"""

import sys

sys.path.insert(0, "/opt/trn_rl_repo")

import ml_dtypes
import numpy as np

import concourse.bass as bass  # noqa: F401
import concourse.mybir as mybir
import concourse.tile as tile  # noqa: F401
from concourse import bacc
from concourse.bass_utils import run_bass_kernel_spmd

P = 128
B, N, D = 4, 2048, 768
H, DH = 12, 64
DC = D // P              # 6 contraction chunks
SCALE = DH ** -0.5       # 0.125
MASK_NEG = -30000.0

f32 = mybir.dt.float32
bf16 = mybir.dt.bfloat16
np_bf16 = ml_dtypes.bfloat16

_BUILD_CACHE = {}


def build(njt: int, niq: int) -> "bacc.Bacc":
    """Build the SPMD program. njt = key tiles containing any unmasked key;
    niq = query tiles needed for this core's share of unmasked queries."""
    key = (njt, niq)
    if key in _BUILD_CACHE:
        return _BUILD_CACHE[key]

    nk = njt * P             # active key columns
    nq = niq * P             # query rows computed on this core
    # free-dim chunking of the query axis (PSUM bank = 512 fp32)
    qch = [(off, min(512, nq - off)) for off in range(0, nq, 512)]

    nc = bacc.Bacc()
    xkT_d = nc.declare_dram_parameter("xkT", [D, nk], bf16, isOutput=False)
    xqT_d = nc.declare_dram_parameter("xqT", [D, nq], bf16, isOutput=False)
    wq_d = nc.declare_dram_parameter("Wq", [D, D], bf16, isOutput=False)
    wk_d = nc.declare_dram_parameter("Wk", [D, D], bf16, isOutput=False)
    wv_d = nc.declare_dram_parameter("Wv", [D, D], bf16, isOutput=False)
    wo_d = nc.declare_dram_parameter("Wo", [D, D], bf16, isOutput=False)
    # cmnegT[p, t] = 0.0 if key (t*128+p) unmasked else -30000.0
    cmneg_d = nc.declare_dram_parameter("cmnegT", [P, njt], f32, isOutput=False)
    out_d = nc.declare_dram_parameter("out", [nq, D], f32, isOutput=True)

    xkT_r = xkT_d.rearrange("(c p) n -> p c n", p=P)
    xqT_r = xqT_d.rearrange("(c p) n -> p c n", p=P)
    wv_r = wv_d.rearrange("(c p) e -> p c e", p=P)
    wq_r = wq_d.rearrange("(c p) e -> p c e", p=P)
    wk_r = wk_d.rearrange("(c p) e -> p c e", p=P)
    wo_r = wo_d.rearrange("(c p) e -> p c e", p=P)

    with tile.TileContext(nc) as tc:
        with tc.tile_pool(name="persist", bufs=1) as persist:
            cmneg = persist.tile([P, njt], f32)
            nc.sync.dma_start(out=cmneg, in_=cmneg_d.ap())
            ones_b = persist.tile([P, H], bf16)
            nc.vector.memset(ones_b, 1.0)

            qT = persist.tile([P, DC, nq], bf16)
            kT = persist.tile([P, DC, nk], bf16)
            vaug = persist.tile([P, njt, H, DH + 2], bf16)
            attnT = persist.tile([P, DC, nq], bf16)
            wo_sb = persist.tile([P, DC, D], bf16)

            with tc.tile_pool(name="xk_pool", bufs=1) as xk_pool:
                xkT = xk_pool.tile([P, DC, nk], bf16)
                vproj_scope = nc.named_scope("vproj"); vproj_scope.__enter__()
                # ------------- phase 1: V projection -------------
                with tc.tile_pool(name="wv_pool", bufs=1) as wv_pool, \
                     tc.tile_pool(name="psp2", bufs=2, space="PSUM") as psp2:
                    wv_sb = wv_pool.tile([P, DC, D], bf16)
                    for dc in range(DC):
                        nc.sync.dma_start(out=wv_sb[:, dc, :], in_=wv_r[:, dc, :])
                    for cg in range(0, nk, 512):
                        ce = min(cg + 512, nk)
                        for dc in range(DC):
                            nc.sync.dma_start(
                                out=xkT[:, dc, cg:ce], in_=xkT_r[:, dc, cg:ce]
                            )
                    for jt in range(njt):
                        psv = psp2.tile([P, D], f32, tag="psv")
                        for half, (lo, hi) in enumerate(((0, 512), (512, 768))):
                            for dc in range(DC):
                                nc.tensor.matmul(
                                    psv[:, lo:hi],
                                    xkT[:, dc, jt * P : (jt + 1) * P],
                                    wv_sb[:, dc, lo:hi],
                                    start=(dc == 0),
                                    stop=(dc == DC - 1),
                                )
                        nc.vector.tensor_copy(
                            vaug[:, jt, :, 0:DH],
                            psv.rearrange("p (h d) -> p h d", h=H),
                        )
                        nc.vector.tensor_copy(
                            vaug[:, jt, :, DH : DH + 2],
                            ones_b[:, :, None].to_broadcast([P, H, 2]),
                        )

                vproj_scope.__exit__(None, None, None)
                qproj_scope = nc.named_scope("qproj"); qproj_scope.__enter__()
                # ---------------- phase 2: Q projection ----------------
                with tc.tile_pool(name="xq_pool", bufs=1) as xq_pool, \
                     tc.tile_pool(name="wst1", bufs=2) as wst1, \
                     tc.tile_pool(name="psp1", bufs=2, space="PSUM") as psp1:
                    xqT = xq_pool.tile([P, DC, nq], bf16)
                    for dc in range(DC):
                        nc.sync.dma_start(out=xqT[:, dc, :], in_=xqT_r[:, dc, :])
                    for hdt in range(DC):
                        wq_t = wst1.tile([P, DC, P], bf16, tag="wstream")
                        nc.sync.dma_start(
                            out=wq_t, in_=wq_r[:, :, hdt * P : (hdt + 1) * P]
                        )
                        ps = psp1.tile([P, nq], f32, tag="psproj")
                        for off, sz in qch:
                            for dc in range(DC):
                                nc.tensor.matmul(
                                    ps[:, off : off + sz],
                                    wq_t[:, dc, :],
                                    xqT[:, dc, off : off + sz],
                                    start=(dc == 0),
                                    stop=(dc == DC - 1),
                                )
                        nc.vector.tensor_copy(qT[:, hdt, :], ps)

                qproj_scope.__exit__(None, None, None)
                kproj_scope = nc.named_scope("kproj"); kproj_scope.__enter__()
                # ---------------- phase 3: K projection ----------------
                with tc.tile_pool(name="wst3", bufs=2) as wst3, \
                     tc.tile_pool(name="psp3", bufs=2, space="PSUM") as psp3:
                    kch = [(off, min(512, nk - off)) for off in range(0, nk, 512)]
                    for hdt in range(DC):
                        wk_t = wst3.tile([P, DC, P], bf16, tag="wstream3")
                        nc.sync.dma_start(
                            out=wk_t, in_=wk_r[:, :, hdt * P : (hdt + 1) * P]
                        )
                        ps = psp3.tile([P, nk], f32, tag="psproj3")
                        for off, sz in kch:
                            for dc in range(DC):
                                nc.tensor.matmul(
                                    ps[:, off : off + sz],
                                    wk_t[:, dc, :],
                                    xkT[:, dc, off : off + sz],
                                    start=(dc == 0),
                                    stop=(dc == DC - 1),
                                )
                        nc.vector.tensor_copy(kT[:, hdt, :], ps)

            kproj_scope.__exit__(None, None, None)
            # Wo loads during the attention phase (needed only at the end)
            for dc in range(DC):
                nc.sync.dma_start(out=wo_sb[:, dc, :], in_=wo_r[:, dc, :])
            attn_scope = nc.named_scope("attn"); attn_scope.__enter__()
            # ---------------- phase 4: attention heads ----------------
            with tc.tile_pool(name="psS", bufs=2, space="PSUM") as psS_pool, \
                 tc.tile_pool(name="psO", bufs=2, space="PSUM") as psO_pool, \
                 tc.tile_pool(name="pts", bufs=2) as pts, \
                 tc.tile_pool(name="nrm", bufs=2) as nrm:
                for h in range(H):
                    hdt, hh = h // 2, h % 2
                    pbase = DH * hh
                    psO = psO_pool.tile([DH + 2, nq], f32, tag="psO",
                                        name=f"psO{h % 2}")
                    prev = None
                    for jt in range(njt + 1):
                        cur = None
                        if jt < njt:
                            psS = psS_pool.tile([P, nq], f32, tag="psS",
                                                name=f"psS{jt % 2}")
                            for off, sz in qch:
                                nc.tensor.matmul(
                                    psS[:, off : off + sz],
                                    kT[pbase : pbase + DH, hdt,
                                       jt * P : (jt + 1) * P],
                                    qT[pbase : pbase + DH, hdt, off : off + sz],
                                    start=True,
                                    stop=True,
                                )
                            cur = psS
                        if prev is not None:
                            pjt, pT = prev
                            for off, sz in qch:
                                nc.tensor.matmul(
                                    psO[:, off : off + sz],
                                    vaug[:, pjt, h, :],
                                    pT[:, off : off + sz],
                                    start=(pjt == 0),
                                    stop=(pjt == njt - 1),
                                )
                        if cur is not None:
                            pT = pts.tile([P, nq], bf16, tag="pT")
                            nc.scalar.activation(
                                pT,
                                cur,
                                mybir.ActivationFunctionType.Exp,
                                bias=cmneg[:, jt : jt + 1],
                                scale=SCALE,
                            )
                            prev = (jt, pT)
                    # normalize: 1/s on DVE, broadcast on GpSimd, multiply
                    # while copying out of PSUM (inputs share start part. 0)
                    r_row = nrm.tile([1, nq], f32, tag="r_row")
                    nc.vector.reciprocal(r_row, psO[DH : DH + 1, :])
                    rb_sb = nrm.tile([DH, nq], f32, tag="rb_sb")
                    nc.gpsimd.partition_broadcast(rb_sb, r_row, channels=DH)
                    nc.vector.tensor_mul(
                        attnT[pbase : pbase + DH, hdt, :],
                        psO[0:DH, :],
                        rb_sb,
                    )
            attn_scope.__exit__(None, None, None)
            fin_scope = nc.named_scope("final"); fin_scope.__enter__()
            # ---------------- phase 5: output projection ----------------
            with tc.tile_pool(name="fin", bufs=2) as fin, \
                 tc.tile_pool(name="psF", bufs=2, space="PSUM") as psF_pool:
                for it in range(niq):
                    psF = psF_pool.tile([P, D], f32, tag="psF")
                    for lo, hi in ((0, 512), (512, 768)):
                        for c in range(DC):
                            nc.tensor.matmul(
                                psF[:, lo:hi],
                                attnT[:, c, it * P : (it + 1) * P],
                                wo_sb[:, c, lo:hi],
                                start=(c == 0),
                                stop=(c == DC - 1),
                            )
                    out_sb = fin.tile([P, D], f32, tag="outsb")
                    nc.vector.tensor_copy(out_sb, psF)
                    nc.sync.dma_start(
                        out=out_d.ap()[it * P : (it + 1) * P, :], in_=out_sb
                    )
            fin_scope.__exit__(None, None, None)

    nc.compile()
    _BUILD_CACHE[key] = nc
    return nc


def _marshal(x, x_mask, Wq, Wk, Wv, Wo):
    """Build per-core input maps. Returns (in_maps, njt, niq, scatter)."""
    x = np.asarray(x, dtype=np.float32)
    x_mask = np.asarray(x_mask).astype(bool)
    Wb = {}
    for name, W in (("Wq", Wq), ("Wk", Wk), ("Wv", Wv), ("Wo", Wo)):
        Wb[name] = np.ascontiguousarray(
            np.asarray(W, dtype=np.float32).astype(np_bf16)
        )

    korders, kcounts, urows = [], [], []
    qidx_all = []
    for b in range(B):
        korders.append(np.argsort(~x_mask[b], kind="stable"))
        kcounts.append(int(x_mask[b].sum()))
        # uniform-softmax row for masked queries: mean over ALL keys
        mv = (x[b].mean(0) @ np.asarray(Wv, dtype=np.float32))
        urows.append(mv @ np.asarray(Wo, dtype=np.float32))
        qidx_all.append(np.nonzero(x_mask[b])[0])

    njt = max(1, -(-max(kcounts) // P))
    nk = njt * P

    # split each batch's unmasked queries between its two cores
    qsplit = []
    for b in range(B):
        qa = qidx_all[b]
        half = (len(qa) + 1) // 2
        qsplit.append((qa[:half], qa[half:]))
    niq = max(1, -(-max(len(qs[i]) for qs in qsplit for i in (0, 1)) // P))
    nq = niq * P

    in_maps = []
    scatter = []   # per core: (b, q_indices)
    for c in range(8):
        b, qh = c // 2, c % 2
        order = korders[b][:nk]
        qa = qsplit[b][qh]
        pad = np.zeros(nq - len(qa), dtype=qa.dtype)  # row 0 dup, discarded
        qfull = np.concatenate([qa, pad])

        xT = x[b].T  # [768, 2048] view
        cm = np.where(x_mask[b][order], 0.0, MASK_NEG).astype(np.float32)

        in_maps.append({
            "xkT": np.ascontiguousarray(xT[:, order].astype(np_bf16)),
            "xqT": np.ascontiguousarray(xT[:, qfull].astype(np_bf16)),
            "Wq": Wb["Wq"], "Wk": Wb["Wk"], "Wv": Wb["Wv"], "Wo": Wb["Wo"],
            "cmnegT": np.ascontiguousarray(cm.reshape(njt, P).T),
        })
        scatter.append((b, qa))
    return in_maps, njt, niq, scatter, urows


def run(x, x_mask, Wq, Wk, Wv, Wo, trace=False, tmpdir=None):
    """Run on 8 cores; returns (full_output, BassKernelResults)."""
    in_maps, njt, niq, scatter, urows = _marshal(x, x_mask, Wq, Wk, Wv, Wo)
    nc = build(njt, niq)
    res = run_bass_kernel_spmd(
        nc, in_maps, core_ids=list(range(8)), trace=trace, tmpdir=tmpdir
    )
    x_mask = np.asarray(x_mask).astype(bool)
    out = np.empty((B, N, D), dtype=np.float32)
    for b in range(B):
        out[b, ~x_mask[b]] = urows[b]
    for c in range(8):
        b, qa = scatter[c]
        out[b, qa] = res.results[c]["out"][: len(qa)]
    return out, res


def kernel(**inputs) -> np.ndarray:
    out, _ = run(
        inputs["x"], inputs["x_mask"],
        inputs["Wq"], inputs["Wk"], inputs["Wv"], inputs["Wo"],
        trace=False,
    )
    return out
